# revision 16
# baseline (speedup 1.0000x reference)
"""Trainium2 Bass kernel for AnchorPositionalEncoding.

Reference computation (single device):
    deg = sum(adj, axis=-1)                    # [N]
    nrm = ||deg||_2 + 1e-6
    sim = outer(deg, deg[:A]) / nrm            # [N, A]
    out = softmax(sim, axis=-1) @ anchor_emb   # [N, H]

Distribution: adj is sharded row-wise across 8 NeuronCores ([N/8, N] each).
Phase 1 is a pure streaming row-reduce of the 128 MB shard (memory-bound,
DMA-paced; the vector engine hides under the DMA).  At the end, ONE tiny
AllGather ships [per-partition sumsq partials (128) | local deg[0:64]]
per core; its inputs are first transposed to contiguous rows on the PE
(a partition-strided 128x4B DMA to HBM costs ~12 us in sub-512B read-
modify-write transactions; a contiguous 512 B row costs ~1 us).

Epilogue: the local deg row is flattened to partition 0 (SBUF->SBUF DMA,
overlapping the AllGather) and partition-broadcast to 64 rows (gpsimd,
also during the AllGather).  After the gather:
    simT[a, q] = deg64_global[a] * deg_local[q]   one DVE tensor_scalar
    eT   = exp(simT * (1/nrm) - 64)               one ACT op
    po   = eT.T @ [anchor_emb | ones]             16 PE matmuls -> row sums
    out  = po[:, :H] * 1/po[:, H]                 ACT copy, scale=recip

Numerics: softmax logits are deg_p * deg_a / ||deg|| ~= sqrt(N)/2 = 64
for uniform adj, so instead of a per-row max subtraction we shift by a
constant -64 before exp (softmax is shift-invariant; keeps the exp
argument near 0 where the ACT table is accurate, far from f32 overflow).
The reference's +1e-6 on ||deg|| (~1e6 here) is below f32 resolution of
the norm, so it is not applied.
"""

import numpy as np

from concourse import bass, bacc, mybir, tile, bass_utils, masks

N = 16384          # graph nodes
NCORES = 8
SHARD = N // NCORES  # 2048 rows per core
A = 64             # num anchors
H = 128            # hidden dim
P = 128            # SBUF partitions
NT = SHARD // P    # 16 row tiles per core
CHUNK = 4096       # free-dim chunk for the streaming reduce
CHUNK_BUFS = 8
N_QUEUES = 2       # 1: all loads on sync; 2: alternate sync/scalar HWDGE rings
F32 = mybir.dt.float32
AX = mybir.AxisListType
AF = mybir.ActivationFunctionType
LOGIT_SHIFT = -64.0


def build_nc():
    nch = N // CHUNK  # chunks per row tile
    nc = bacc.Bacc(
        "TRN2", target_bir_lowering=False, debug=False, num_devices=NCORES
    )
    adj = nc.dram_tensor("adj", [SHARD, N], F32, kind="ExternalInput")
    emb_d = nc.dram_tensor("anchor_emb", [A, H], F32, kind="ExternalInput")
    out_d = nc.dram_tensor("out", [SHARD, H], F32, kind="ExternalOutput")

    with tile.TileContext(nc) as tc:
        with (
            tc.tile_pool(name="const", bufs=1) as const,
            tc.tile_pool(name="chunks", bufs=CHUNK_BUFS) as chunks,
            tc.tile_pool(name="stats", bufs=1) as stats,
            tc.tile_pool(name="work", bufs=2) as work,
            tc.tile_pool(name="psum_s", bufs=1, space="PSUM") as psum_s,
            tc.tile_pool(name="psum", bufs=2, space="PSUM") as psum,
            tc.tile_pool(name="dram", bufs=1, space="DRAM") as dram,
        ):
            ident = const.tile([P, P], F32)
            masks.make_identity(nc, ident[:])
            ones_col = const.tile([P, 1], F32)
            nc.gpsimd.memset(ones_col[:], 1.0)
            shift = const.tile([P, 1], F32)
            nc.gpsimd.memset(shift[:], LOGIT_SHIFT)
            # anchor_emb augmented with a ones column: the anchor matmul
            # then also produces the softmax denominator.
            embA = const.tile([A, H + 1], F32)
            nc.sync.dma_start(embA[:, 0:H], emb_d[:])
            nc.gpsimd.memset(embA[:, H : H + 1], 1.0)

            # deg columns 0..NT-1 plus a sumsq-partials column NT
            degs = stats.tile([P, NT + 1], F32)

            # collective payload per core: [sumsq partials (P) | deg[0:A]]
            CCW = P + A
            cc_in = dram.tile([CCW], F32)
            g = dram.tile([NCORES * CCW], F32, addr_space="Shared")

            # ---- phase 1: stream adj, reduce rows -------------------------
            partials = stats.tile([P, NT * nch], F32)
            dma_engines = [nc.sync, nc.scalar][:N_QUEUES]
            for t in range(NT):
                for c in range(nch):
                    ch = chunks.tile([P, CHUNK], F32)
                    k = t * nch + c
                    dma_engines[k % len(dma_engines)].dma_start(
                        ch[:],
                        adj[t * P : (t + 1) * P, c * CHUNK : (c + 1) * CHUNK],
                    )
                    nc.vector.reduce_sum(partials[:, k : k + 1], ch[:], axis=AX.X)
                nc.vector.reduce_sum(
                    degs[:, t : t + 1],
                    partials[:, t * nch : (t + 1) * nch],
                    axis=AX.X,
                )

            # sumsq partials into column NT, then transpose all stats to rows
            sq = stats.tile([P, NT], F32)
            nc.vector.tensor_mul(sq[:], degs[:, 0:NT], degs[:, 0:NT])
            nc.vector.reduce_sum(degs[:, NT : NT + 1], sq[:], axis=AX.X)
            pdegT = psum_s.tile([NT + 1, P], F32)
            nc.tensor.transpose(pdegT[:], degs[:], ident[:])
            degT = stats.tile([NT + 1, P], F32)  # row t = deg of row tile t
            nc.scalar.copy(degT[:], pdegT[:])

            # ---- one tiny AllGather: [sumsq partials | local deg[0:64]] ---
            # (contiguous-row sources; gpsimd-issued so the collective
            # trigger that follows on the same engine fires immediately)
            nc.gpsimd.dma_start(cc_in[0:P], degT[NT : NT + 1, :])
            nc.gpsimd.dma_start(cc_in[P:CCW], degT[0:1, 0:A])
            nc.gpsimd.collective_compute(
                "AllGather",
                mybir.AluOpType.bypass,
                replica_groups=[list(range(NCORES))],
                ins=[cc_in[:].opt()],
                outs=[g[:].opt()],
            )

            # local deg as a single partition-0 row, broadcast to A rows —
            # both independent of the collective, so they overlap it.
            deg_row = stats.tile([1, SHARD], F32)
            nc.sync.dma_start(deg_row[:], degT[0:NT, :])
            db = stats.tile([A, SHARD], F32)
            nc.gpsimd.partition_broadcast(db[:], deg_row[:])

            # total sumsq -> nrm -> 1/nrm
            sq8 = stats.tile([NCORES, P], F32)
            nc.sync.dma_start(sq8[:], g[:].rearrange("(r k) -> r k", k=CCW)[:, 0:P])
            d64c = stats.tile([A, 1], F32)  # core 0's deg[0:64], on partitions
            nc.sync.dma_start(d64c[:], g[:].rearrange("(r k) -> r k", k=CCW)[0:1, P:CCW])
            s8 = stats.tile([NCORES, 1], F32)
            nc.vector.reduce_sum(s8[:], sq8[:], axis=AX.X)
            pred = psum_s.tile([1, 1], F32)
            nc.tensor.matmul(
                pred[:], s8[:], ones_col[0:NCORES, 0:1], start=True, stop=True
            )
            nrm = stats.tile([1, 1], F32)
            nc.scalar.activation(nrm[:], pred[:], AF.Sqrt)
            # re-warm the Exp table immediately after Sqrt evicted it, while
            # the reciprocal/broadcast steps below run on other engines
            warm = stats.tile([1, 1], F32)
            nc.scalar.activation(warm[:], ones_col[0:1, 0:1], AF.Exp)
            inv = stats.tile([1, 1], F32)
            nc.vector.reciprocal(inv[:], nrm[:])
            invA = stats.tile([A, 1], F32)
            nc.gpsimd.partition_broadcast(invA[:], inv[:])

            # ---- phase 2: one fused sim/exp, then per-tile anchor matmul --
            simT = stats.tile([A, SHARD], F32)
            nc.vector.tensor_scalar_mul(simT[:], db[:], d64c[:, 0:1])
            eT = stats.tile([A, SHARD], F32)
            nc.scalar.activation(
                eT[:], simT[:], AF.Exp,
                bias=shift[0:A, 0:1], scale=invA[:, 0:1],
            )
            o_all = stats.tile([P, NT * H], F32)
            for t in range(NT):
                po = psum.tile([P, H + 1], F32)
                nc.tensor.matmul(
                    po[:], eT[:, t * P : (t + 1) * P], embA[:],
                    start=True, stop=True,
                )
                r_t = work.tile([P, 1], F32)
                nc.vector.reciprocal(r_t[:], po[:, H : H + 1])
                nc.scalar.activation(
                    o_all[:, t * H : (t + 1) * H], po[:, 0:H], AF.Copy,
                    bias=0.0, scale=r_t[:, 0:1],
                )
                nc.sync.dma_start(
                    out_d[t * P : (t + 1) * P, :],
                    o_all[:, t * H : (t + 1) * H],
                )

    nc.compile()
    return nc


_NC_CACHE = None


def _get_nc():
    global _NC_CACHE
    if _NC_CACHE is None:
        _NC_CACHE = build_nc()
    return _NC_CACHE


def _in_maps(adj, anchor_emb):
    adj = np.ascontiguousarray(adj, dtype=np.float32)
    anchor_emb = np.ascontiguousarray(anchor_emb, dtype=np.float32)
    return [
        {
            "adj": np.ascontiguousarray(adj[i * SHARD : (i + 1) * SHARD, :]),
            "anchor_emb": anchor_emb,
        }
        for i in range(NCORES)
    ]


def run(adj, anchor_emb, **kwargs):
    nc = _get_nc()
    res = bass_utils.run_bass_kernel_spmd(
        nc, _in_maps(adj, anchor_emb), core_ids=list(range(NCORES)), **kwargs
    )
    out = np.concatenate(
        [res.results[i]["out"] for i in range(NCORES)], axis=0
    ).astype(np.float32)
    return out, res


def kernel(adj, anchor_emb):
    out, _ = run(adj, anchor_emb)
    return out


# revision 19
# speedup vs baseline: 1.0442x; 1.0442x over previous
"""Trainium2 Bass kernel for AnchorPositionalEncoding.

Reference computation (single device):
    deg = sum(adj, axis=-1)                    # [N]
    nrm = ||deg||_2 + 1e-6
    sim = outer(deg, deg[:A]) / nrm            # [N, A]
    out = softmax(sim, axis=-1) @ anchor_emb   # [N, H]

Distribution: adj is sharded row-wise across 8 NeuronCores ([N/8, N] each).
Phase 1 is a pure streaming row-reduce of the 128 MB shard (memory-bound,
DMA-paced; the vector engine hides under the DMA).  At the end, ONE tiny
AllGather ships [per-partition sumsq partials (128) | local deg[0:64]]
per core; its inputs are first transposed to contiguous rows on the PE
(a partition-strided 128x4B DMA to HBM costs ~12 us in sub-512B read-
modify-write transactions; a contiguous 512 B row costs ~1 us).

Epilogue: the local deg row is flattened to partition 0 (SBUF->SBUF DMA,
overlapping the AllGather) and partition-broadcast to 64 rows (gpsimd,
also during the AllGather).  After the gather:
    simT[a, q] = deg64_global[a] * deg_local[q]   one DVE tensor_scalar
    eT   = exp(simT * (1/nrm) - 64)               one ACT op
    po   = eT.T @ [anchor_emb | ones]             16 PE matmuls -> row sums
    out  = po[:, :H] * 1/po[:, H]                 ACT copy, scale=recip

Numerics: softmax logits are deg_p * deg_a / ||deg|| ~= sqrt(N)/2 = 64
for uniform adj, so instead of a per-row max subtraction we shift by a
constant -64 before exp (softmax is shift-invariant; keeps the exp
argument near 0 where the ACT table is accurate, far from f32 overflow).
The reference's +1e-6 on ||deg|| (~1e6 here) is below f32 resolution of
the norm, so it is not applied.
"""

import numpy as np

from concourse import bass, bacc, mybir, tile, bass_utils, masks

N = 16384          # graph nodes
NCORES = 8
SHARD = N // NCORES  # 2048 rows per core
A = 64             # num anchors
H = 128            # hidden dim
P = 128            # SBUF partitions
NT = SHARD // P    # 16 row tiles per core
CHUNK = 2048       # free-dim chunk for the streaming reduce
CHUNK_BUFS = 16
N_QUEUES = 2       # 1: all loads on sync; 2: alternate sync/scalar HWDGE rings
F32 = mybir.dt.float32
AX = mybir.AxisListType
AF = mybir.ActivationFunctionType
LOGIT_SHIFT = -64.0


def build_nc():
    nch = N // CHUNK  # chunks per row tile
    nc = bacc.Bacc(
        "TRN2", target_bir_lowering=False, debug=False, num_devices=NCORES
    )
    adj = nc.dram_tensor("adj", [SHARD, N], F32, kind="ExternalInput")
    emb_d = nc.dram_tensor("anchor_emb", [A, H], F32, kind="ExternalInput")
    out_d = nc.dram_tensor("out", [SHARD, H], F32, kind="ExternalOutput")

    with tile.TileContext(nc) as tc:
        with (
            tc.tile_pool(name="const", bufs=1) as const,
            tc.tile_pool(name="chunks", bufs=CHUNK_BUFS) as chunks,
            tc.tile_pool(name="stats", bufs=1) as stats,
            tc.tile_pool(name="work", bufs=2) as work,
            tc.tile_pool(name="psum_s", bufs=1, space="PSUM") as psum_s,
            tc.tile_pool(name="psum", bufs=2, space="PSUM") as psum,
            tc.tile_pool(name="dram", bufs=1, space="DRAM") as dram,
        ):
            ident = const.tile([P, P], F32)
            masks.make_identity(nc, ident[:])
            ones_col = const.tile([P, 1], F32)
            nc.gpsimd.memset(ones_col[:], 1.0)
            shift = const.tile([P, 1], F32)
            nc.gpsimd.memset(shift[:], LOGIT_SHIFT)
            # anchor_emb augmented with a ones column: the anchor matmul
            # then also produces the softmax denominator.
            embA = const.tile([A, H + 1], F32)
            nc.gpsimd.dma_start(embA[:, 0:H], emb_d[:])
            nc.gpsimd.memset(embA[:, H : H + 1], 1.0)

            # deg columns 0..NT-1 plus a sumsq-partials column NT
            degs = stats.tile([P, NT + 1], F32)

            # collective payload per core: [sumsq partials (P) | deg[0:A]]
            CCW = P + A
            cc_in = dram.tile([CCW], F32)
            g = dram.tile([NCORES * CCW], F32, addr_space="Shared")

            # ---- phase 1: stream adj, reduce rows -------------------------
            partials = stats.tile([P, NT * nch], F32)
            dma_engines = [nc.sync, nc.scalar][:N_QUEUES]
            for t in range(NT):
                for c in range(nch):
                    ch = chunks.tile([P, CHUNK], F32)
                    k = t * nch + c
                    dma_engines[k % len(dma_engines)].dma_start(
                        ch[:],
                        adj[t * P : (t + 1) * P, c * CHUNK : (c + 1) * CHUNK],
                    )
                    nc.vector.reduce_sum(partials[:, k : k + 1], ch[:], axis=AX.X)
                nc.vector.reduce_sum(
                    degs[:, t : t + 1],
                    partials[:, t * nch : (t + 1) * nch],
                    axis=AX.X,
                )

            # sumsq partials into column NT, then transpose all stats to rows
            sq = stats.tile([P, NT], F32)
            nc.vector.tensor_mul(sq[:], degs[:, 0:NT], degs[:, 0:NT])
            nc.vector.reduce_sum(degs[:, NT : NT + 1], sq[:], axis=AX.X)
            pdegT = psum_s.tile([NT + 1, P], F32)
            nc.tensor.transpose(pdegT[:], degs[:], ident[:])
            degT = stats.tile([NT + 1, P], F32)  # row t = deg of row tile t
            nc.scalar.copy(degT[:], pdegT[:])

            # ---- one tiny AllGather: [sumsq partials | local deg[0:64]] ---
            # (contiguous-row sources; gpsimd-issued so the collective
            # trigger that follows on the same engine fires immediately)
            nc.gpsimd.dma_start(cc_in[0:P], degT[NT : NT + 1, :])
            nc.gpsimd.dma_start(cc_in[P:CCW], degT[0:1, 0:A])
            nc.gpsimd.collective_compute(
                "AllGather",
                mybir.AluOpType.bypass,
                replica_groups=[list(range(NCORES))],
                ins=[cc_in[:].opt()],
                outs=[g[:].opt()],
            )

            # local deg as a single partition-0 row, broadcast to A rows —
            # both independent of the collective, so they overlap it.
            deg_row = stats.tile([1, SHARD], F32)
            nc.sync.dma_start(deg_row[:], degT[0:NT, :])
            db = stats.tile([A, SHARD], F32)
            nc.gpsimd.partition_broadcast(db[:], deg_row[:])

            # total sumsq -> nrm -> 1/nrm
            sq8 = stats.tile([NCORES, P], F32)
            nc.sync.dma_start(sq8[:], g[:].rearrange("(r k) -> r k", k=CCW)[:, 0:P])
            d64c = stats.tile([A, 1], F32)  # core 0's deg[0:64], on partitions
            nc.sync.dma_start(d64c[:], g[:].rearrange("(r k) -> r k", k=CCW)[0:1, P:CCW])
            s8 = stats.tile([NCORES, 1], F32)
            nc.vector.reduce_sum(s8[:], sq8[:], axis=AX.X)
            pred = psum_s.tile([1, 1], F32)
            nc.tensor.matmul(
                pred[:], s8[:], ones_col[0:NCORES, 0:1], start=True, stop=True
            )
            nrm = stats.tile([1, 1], F32)
            nc.scalar.activation(nrm[:], pred[:], AF.Sqrt)
            # re-warm the Exp table immediately after Sqrt evicted it, while
            # the reciprocal/broadcast steps below run on other engines
            warm = stats.tile([1, 1], F32)
            nc.scalar.activation(warm[:], ones_col[0:1, 0:1], AF.Exp)
            inv = stats.tile([1, 1], F32)
            nc.vector.reciprocal(inv[:], nrm[:])
            invA = stats.tile([A, 1], F32)
            nc.gpsimd.partition_broadcast(invA[:], inv[:])

            # ---- phase 2: one fused sim/exp, then per-tile anchor matmul --
            # simT = db * d64c * (1/nrm), both scalars fused in one DVE op;
            # exp then has no extra dependency and the warm table is ready.
            simT = stats.tile([A, SHARD], F32)
            nc.vector.tensor_scalar(
                simT[:], db[:], d64c[:, 0:1], invA[:, 0:1],
                op0=mybir.AluOpType.mult, op1=mybir.AluOpType.mult,
            )
            eT = stats.tile([A, SHARD], F32)
            nc.scalar.activation(
                eT[:], simT[:], AF.Exp, bias=shift[0:A, 0:1], scale=1.0,
            )
            o_all = stats.tile([P, NT * H], F32)
            for t in range(NT):
                po = psum.tile([P, H + 1], F32)
                nc.tensor.matmul(
                    po[:], eT[:, t * P : (t + 1) * P], embA[:],
                    start=True, stop=True,
                )
                r_t = work.tile([P, 1], F32)
                nc.vector.reciprocal(r_t[:], po[:, H : H + 1])
                nc.scalar.activation(
                    o_all[:, t * H : (t + 1) * H], po[:, 0:H], AF.Copy,
                    bias=0.0, scale=r_t[:, 0:1],
                )
                nc.sync.dma_start(
                    out_d[t * P : (t + 1) * P, :],
                    o_all[:, t * H : (t + 1) * H],
                )

    nc.compile()
    return nc


_NC_CACHE = None


def _get_nc():
    global _NC_CACHE
    if _NC_CACHE is None:
        _NC_CACHE = build_nc()
    return _NC_CACHE


def _in_maps(adj, anchor_emb):
    adj = np.ascontiguousarray(adj, dtype=np.float32)
    anchor_emb = np.ascontiguousarray(anchor_emb, dtype=np.float32)
    return [
        {
            "adj": np.ascontiguousarray(adj[i * SHARD : (i + 1) * SHARD, :]),
            "anchor_emb": anchor_emb,
        }
        for i in range(NCORES)
    ]


def run(adj, anchor_emb, **kwargs):
    nc = _get_nc()
    res = bass_utils.run_bass_kernel_spmd(
        nc, _in_maps(adj, anchor_emb), core_ids=list(range(NCORES)), **kwargs
    )
    out = np.concatenate(
        [res.results[i]["out"] for i in range(NCORES)], axis=0
    ).astype(np.float32)
    return out, res


def kernel(adj, anchor_emb):
    out, _ = run(adj, anchor_emb)
    return out
